# revision 15
# baseline (speedup 1.0000x reference)
"""Distributed LinearAndSoftmax loss kernel for 8 Trainium2 NeuronCores.

Problem: loss = mean_n[ logsumexp_v(x_n . W_v + b_v) - (x_n . W_lab_n + b_lab_n) ]
with x [16,512,768] (N=8192 rows), W [30523,768], b [30523], label [16,512].

Sharding: vocab (tensor-parallel) 8 ways — each core computes partial
sum-exp over its 3840-column vocab shard (padded 30523 -> 30720, pad bias
= -30000 so exp == 0) for ALL 8192 rows; label-logit dot is data-parallel
(1024 rows/core). Cross-shard combine (8 x [8192] f32) happens on host.

v2 design (vs the 877us fp32r baseline): weight-stationary fp8 DoubleRow.
  - Transposed layout: logits come out as [vocab_tile(128 part), rows(free)],
    W-block stationary in the PE, x^T streams as the moving operand. Each
    stationary is reused for RC=4 moving matmuls (amortizes LDWEIGHTS,
    which the baseline paid serially per-MM: 320ns vs 213ns theoretical).
  - fp8e4 (e4m3, TRN max +-240) inputs scaled by SX=16 (x) / SW=4096 (W)
    to dodge the subnormal range; DoubleRow packs 2 fp8 weights per PE
    cell -> K=256 per matmul, 2x bf16 FLOP rate. Logit descale 1/(SX*SW)
    is folded into the ACT exp's free affine (scale), bias b_v folded in
    as the per-partition ACT bias -> no separate bias add at all.
  - Per (row-group, v_tile): 12 DR matmuls into a 4-bank PSUM tile
    (double-buffered), one big ACT exp [128,2048] PSUM->SBUF, one DVE
    add into the per-rowgroup accumulator. Cross-partition (vocab) final
    reduction via a ones-vector f32r matmul at the end (tiny).
CPU-sim accuracy of this exact quantization: rel err 2.3e-5 (tol 2e-2).
"""

import numpy as np
import concourse.bacc as bacc
import concourse.mybir as mybir
import concourse.tile as tile
from concourse.bass_utils import run_bass_kernel_spmd

F32 = mybir.dt.float32
F32R = mybir.dt.float32r
BF16 = mybir.dt.bfloat16
FP8 = mybir.dt.float8e4
AX = mybir.AxisListType
ALU = mybir.AluOpType
ACTF = mybir.ActivationFunctionType
DR = mybir.MatmulPerfMode.DoubleRow

B, S, D, V = 16, 512, 768, 30523
N = B * S                  # 8192 rows
NCORES = 8
VP_TOT = 30720             # padded vocab
VP = VP_TOT // NCORES      # 3840 per core
VT = VP // 128             # 30 vocab tiles per core
KS = D // 128              # 6 contraction subtiles of 128
G = KS // 2                # 3 DoubleRow k-pair groups (256 each)
RGN = 4                    # row groups
RC = 4                     # 512-row chunks per row group
RROWS = RC * 512           # 2048 rows per group
LT = N // NCORES // 128    # 8 label row tiles per core

SX = 16.0                  # x fp8 pre-scale
SW = 4096.0                # W fp8 pre-scale
SINV = 1.0 / (SX * SW)     # logit descale, folded into ACT affine
PAD_BIAS = -30000.0

MODE = "fp8dr"             # "fp8dr" | "bf16"
REPEAT = 1


def _dedup_ldweights(nc):
    """Remove Ldweights whose weights AP equals the previously loaded one.

    The tile layer emits one Ldweights per Matmult even when consecutive
    matmuls share the stationary operand; the PE then reloads identical
    weights serially (~200ns each). Equal-AP loads with no sync info are
    no-ops — delete them so the 4 matmuls per (v_tile, k-group) reuse the
    loaded weights. Tracks the PE stream per basic block (weights only
    change via Ldweights/Matmult; other engines don't touch PE state).
    """
    removed = 0
    for f in nc.m.functions:
        for blk in f.blocks:
            il = blk.instructions
            last_sig = None
            i = 0
            while i < len(il):
                inst = il[i]
                op = inst.opcode
                if op == "Ldweights":
                    si = inst.sync_info
                    clean = si is None or (not si.on_wait and not si.on_update)
                    sig = (str(inst.ins[0]), str(inst.perf_mode))
                    if clean and sig == last_sig:
                        del il[i]
                        removed += 1
                        continue
                    last_sig = sig
                i += 1
    return removed


def _thin_pe_sem_incs(nc, sem_name="PE_49"):
    """Drop sem-incs from Matmults nobody waits on.

    Every Matmult posts a serializing EVT_SEM register write (~26ns each,
    per the TRN2 docs); only ~1/6 of the counts are ever awaited. Keep an
    inc exactly at each awaited ordinal and renumber every wait to its
    rank among kept incs. All waits on the PE semaphore are static
    sem-ge-imm, so the mapping is a pure renumbering.
    """
    order = []          # (inst, ordinal) for MMs that inc sem_name
    required = set()
    for f in nc.m.functions:
        for blk in f.blocks:
            for inst in blk.instructions:
                si = inst.sync_info
                if not si:
                    continue
                for u in si.on_update or []:
                    if u.ant_name == sem_name:
                        assert u.update_mode == "sem-inc" and u.update_value == 1, (
                            u.update_mode, u.update_value)
                        order.append(inst)
                for w in si.on_wait or []:
                    if w.ant_name == sem_name:
                        assert w.wait_mode == "sem-ge-imm", w.wait_mode
                        required.add(w.wait_value)
    total = len(order)
    # always keep the final inc so any end-of-program accounting holds
    required.add(total)
    kept_sorted = sorted(k for k in required if 1 <= k <= total)
    rank = {k: i + 1 for i, k in enumerate(kept_sorted)}
    keep_ordinals = set(kept_sorted)
    for ordinal, inst in enumerate(order, start=1):
        if ordinal not in keep_ordinals:
            si = inst.sync_info
            si.on_update = [u for u in si.on_update
                            if u.ant_name != sem_name]
    # renumber waits: wait for ge-K becomes ge-rank(smallest kept >= K)
    import bisect
    for f in nc.m.functions:
        for blk in f.blocks:
            for inst in blk.instructions:
                si = inst.sync_info
                if not si or not si.on_wait:
                    continue
                changed = False
                for w in si.on_wait:
                    if w.ant_name == sem_name:
                        i = bisect.bisect_left(kept_sorted, w.wait_value)
                        w.wait_value = rank[kept_sorted[i]]
                        changed = True
    return total - len(keep_ordinals)


def build(repeat=None, mode=None, ablate=None, dedup=True, semthin=True):
    # ablate: None (full) | "mm" (matmuls only) | "mmact" (no DVE acc)
    #         | "noadd" (DVE copy instead of add)
    mode = mode or MODE
    repeat = repeat or REPEAT
    fp8 = mode.startswith("fp8dr")
    mdt = FP8 if fp8 else BF16
    nc = bacc.Bacc("TRN2", target_bir_lowering=False, debug=False, num_devices=NCORES)
    xq_d = nc.declare_dram_parameter("xq", [128, KS, N], mdt, isOutput=False)
    wq_d = nc.declare_dram_parameter("wq", [128, KS, VP], mdt, isOutput=False)
    bias_d = nc.declare_dram_parameter("biasb", [128, VT], F32, isOutput=False)
    xs_d = nc.declare_dram_parameter("xs", [128, LT, D], BF16, isOutput=False)
    wl_d = nc.declare_dram_parameter("wlab", [128, LT, D], BF16, isOutput=False)
    se_d = nc.declare_dram_parameter("sumexp", [1, N], F32, isOutput=True)
    ld_d = nc.declare_dram_parameter("labdot", [128, LT], F32, isOutput=True)

    with tile.TileContext(nc) as tc:
        with (
            tc.tile_pool(name="const", bufs=1) as constp,
            tc.tile_pool(name="xrp", bufs=2) as xrp,
            tc.tile_pool(name="psum", bufs=2, space="PSUM") as psum,
            tc.tile_pool(name="ep", bufs=3) as ep,
            tc.tile_pool(name="accp", bufs=1) as accp,
            tc.tile_pool(name="labp", bufs=2) as labp,
            tc.tile_pool(name="outp", bufs=1) as outp,
        ):
            wt = constp.tile([128, KS, VP], mdt)
            nc.sync.dma_start(wt[:], wq_d[:])
            biasb = constp.tile([128, VT], F32)
            nc.sync.dma_start(biasb[:], bias_d[:])
            ones = constp.tile([128, 1], BF16)
            nc.vector.memset(ones[:], 1.0)
            ld_all = outp.tile([128, LT], F32)
            se_sb = outp.tile([1, N], F32)
            if ablate in ("mm", "mmact"):
                nc.vector.memset(se_sb[:], 1.0)
                nc.vector.memset(ld_all[:], 0.0)
            accs = [
                accp.tile([128, RROWS], BF16, tag=f"acc{rg}", name=f"acc{rg}")
                for rg in range(RGN)
            ]

            for _ in range(repeat):
                for rg in range(RGN):
                    xr = xrp.tile([128, KS, RROWS], mdt, tag="xr")
                    nc.sync.dma_start(
                        xr[:], xq_d[:, :, rg * RROWS : (rg + 1) * RROWS]
                    )
                    for v in range(VT):
                        pt = psum.tile([128, RROWS], F32, tag="pt")
                        if fp8:
                            for g in range(G):
                                lw = wt[:, 2 * g : 2 * g + 2, v * 128 : (v + 1) * 128]
                                if mode == "fp8dr_ldw":
                                    nc.tensor.ldweights(lw, perf_mode=DR)
                                for rc in range(RC):
                                    nc.tensor.matmul(
                                        pt[:, rc * 512 : (rc + 1) * 512],
                                        lw,
                                        xr[:, 2 * g : 2 * g + 2, rc * 512 : (rc + 1) * 512],
                                        start=(g == 0),
                                        stop=(g == G - 1),
                                        perf_mode=DR,
                                    )
                        else:
                            for ks in range(KS):
                                lw = wt[:, ks, v * 128 : (v + 1) * 128]
                                for rc in range(RC):
                                    nc.tensor.matmul(
                                        pt[:, rc * 512 : (rc + 1) * 512],
                                        lw,
                                        xr[:, ks, rc * 512 : (rc + 1) * 512],
                                        start=(ks == 0),
                                        stop=(ks == KS - 1),
                                    )
                        if ablate == "mm":
                            continue
                        e = ep.tile([128, RROWS], BF16, tag="e")
                        nc.scalar.activation(
                            e[:], pt[:], ACTF.Exp,
                            bias=biasb[:, v : v + 1], scale=SINV,
                        )
                        if ablate == "mmact":
                            continue
                        if v == 0 or ablate == "noadd":
                            nc.vector.tensor_copy(accs[rg][:], e[:])
                        else:
                            nc.vector.tensor_add(accs[rg][:], accs[rg][:], e[:])
                    if rg == 1 and ablate not in ("mm", "mmact"):
                        # label dot-products ride in DVE/DMA slack mid-flight
                        for t in range(LT):
                            xs_t = labp.tile([128, D], BF16, tag="xs")
                            nc.sync.dma_start(xs_t[:], xs_d[:, t])
                            wl_t = labp.tile([128, D], BF16, tag="wl")
                            nc.sync.dma_start(wl_t[:], wl_d[:, t])
                            prod = labp.tile([128, D], F32, tag="prod")
                            nc.vector.tensor_mul(prod[:], xs_t[:], wl_t[:])
                            nc.vector.tensor_reduce(
                                ld_all[:, t : t + 1], prod[:], axis=AX.X, op=ALU.add
                            )
                # cross-partition (vocab-tile) reduction: ones.T @ acc
                for rg in range(RGN if ablate not in ("mm", "mmact") else 0):
                    red = psum.tile([1, RROWS], F32, tag="pt", name=f"red{rg}")
                    for rc in range(RC):
                        nc.tensor.matmul(
                            red[:, rc * 512 : (rc + 1) * 512],
                            ones[:],
                            accs[rg][:, rc * 512 : (rc + 1) * 512],
                            start=True,
                            stop=True,
                        )
                    nc.vector.tensor_copy(
                        se_sb[:, rg * RROWS : (rg + 1) * RROWS], red[:]
                    )
            nc.sync.dma_start(se_d[:], se_sb[:])
            nc.sync.dma_start(ld_d[:], ld_all[:])
    if dedup:
        _dedup_ldweights(nc)
    if semthin:
        _thin_pe_sem_incs(nc)
    nc.compile()
    return nc


def prep_inputs(x, W, b, label, mode=None):
    """Host-side sharding: returns per-core input maps."""
    mode = mode or MODE
    np_dt = mybir.dt.np(FP8 if mode.startswith("fp8dr") else BF16)
    bf16 = mybir.dt.np(BF16)
    sx = SX if mode.startswith("fp8dr") else 1.0
    sw = SW if mode.startswith("fp8dr") else 1.0
    xf = np.ascontiguousarray(np.asarray(x, dtype=np.float32).reshape(N, D))
    W = np.asarray(W, dtype=np.float32)
    b = np.asarray(b, dtype=np.float32)
    lab = np.asarray(label).reshape(N).astype(np.int64)

    Wp = np.zeros((VP_TOT, D), dtype=np.float32)
    Wp[:V] = W
    bp = np.full(VP_TOT, PAD_BIAS, dtype=np.float32)
    bp[:V] = b

    # xq[p, ks, n] = xf[n, ks*128+p] * sx — shared by all cores
    xq = np.ascontiguousarray(
        (xf.T * sx).reshape(KS, 128, N).transpose(1, 0, 2)
    ).astype(np_dt)

    in_maps = []
    for c in range(NCORES):
        Wc = Wp[c * VP : (c + 1) * VP]                      # [VP, D]
        wq = np.ascontiguousarray(
            (Wc.T * sw).reshape(KS, 128, VP).transpose(1, 0, 2)
        ).astype(np_dt)                                     # [128, KS, VP]
        biasb = np.ascontiguousarray(
            bp[c * VP : (c + 1) * VP].reshape(VT, 128).T
        )                                                   # [128, VT]
        rows = slice(c * (N // NCORES), (c + 1) * (N // NCORES))
        xs = np.ascontiguousarray(
            xf[rows].reshape(LT, 128, D).transpose(1, 0, 2)
        ).astype(bf16)
        wlab = np.ascontiguousarray(
            W[lab[rows]].reshape(LT, 128, D).transpose(1, 0, 2)
        ).astype(bf16)
        in_maps.append(
            {"xq": xq, "wq": wq, "biasb": biasb, "xs": xs, "wlab": wlab}
        )
    return in_maps, lab, b


def combine(results, lab, b):
    """Host-side unshard: merge per-core partials into the scalar loss."""
    sumexp = np.zeros(N, dtype=np.float64)
    labdot = np.empty(N, dtype=np.float64)
    for c in range(NCORES):
        sumexp += results[c]["sumexp"].astype(np.float64).reshape(N)
        rows = slice(c * (N // NCORES), (c + 1) * (N // NCORES))
        labdot[rows] = results[c]["labdot"].astype(np.float64).T.reshape(N // NCORES)
    lse = np.log(sumexp)
    nll = lse - (labdot + b.astype(np.float64)[lab])
    return np.asarray(nll.mean(), dtype=np.float32)


def kernel(x, W, b, label):
    in_maps, lab, b32 = prep_inputs(x, W, b, label)
    nc = build()
    res = run_bass_kernel_spmd(nc, in_maps, list(range(NCORES)), trace=False)
    return combine(res.results, lab, b32)
